# revision 8
# baseline (speedup 1.0000x reference)
# SDPA (naive, additive mask) for TRN2, 8 NeuronCores.
#
# Full problem: q/k/v [16, 4096, 64] f32, mask [4096, 4096] f32
#   out = softmax(q @ k^T / 8 + mask) @ v
#
# Sharding (2 head-groups x 4 q-groups = 8 cores, minimizes HBM traffic):
#   core c: hg, qg = divmod(c, 4)
#   heads hg*8:(hg+1)*8, q-rows qg*1024:(qg+1)*1024, k/v full, mask q-slice.
#   Per-core HBM: mask 16MB + K/V 16MB + Q 2MB + out 2MB = 36MB.
#
# Kernel (per core, flash-style with transposed scores):
#   E^T = exp(mask^T) resident in SBUF (bf16)  [t-major so softmax sum is a
#   matmul axis]; per head: scores^T = K^T.T @ Q^T on PE (bf16), exp on ACT
#   (scale=0.125 folded in), multiply by E^T on DVE (all-bf16 2x mode), then
#   PV = [V | ones].T @ attn^T accumulated in PSUM -> unnormalized out^T plus
#   the softmax denominators in the last row; transpose back on PE, scale by
#   reciprocal on DVE, store.

import os
from contextlib import ExitStack

import numpy as np

import concourse.bacc as bacc
import concourse.bass as bass
import concourse.mybir as mybir
import concourse.tile as tile
from concourse import bass2jax
from concourse.masks import make_identity

F32 = mybir.dt.float32
BF = mybir.dt.bfloat16
AF = mybir.ActivationFunctionType

N_CORES = 8
H = 8        # heads per core
SQ = 1024    # q rows per core
SK = 4096    # kv rows
D = 64       # head dim
EB = 3       # t-blocks per exp batch (3 PSUM banks)


def build_bass(H=H, SQ=SQ, SK=SK, D=D, EB=EB) -> bass.Bass:
    TB = SK // 128    # t-blocks
    QB = SQ // 128    # q-blocks of 128
    QW = min(512, SQ)  # q-pass width
    QP = SQ // QW     # q-passes
    QC = QW // 128    # 128-chunks per q-pass
    SCALE = D ** -0.5
    nc = bacc.Bacc("TRN2")
    q_d = nc.dram_tensor("queries", [H, SQ, D], F32, kind="ExternalInput")
    k_d = nc.dram_tensor("keys", [H, SK, D], F32, kind="ExternalInput")
    v_d = nc.dram_tensor("values", [H, SK, D], F32, kind="ExternalInput")
    m_d = nc.dram_tensor("mask", [SQ, SK], F32, kind="ExternalInput")
    o_d = nc.dram_tensor("out", [H, SQ, D], F32, kind="ExternalOutput")

    with tile.TileContext(nc) as tc, ExitStack() as ctx:
        singles = ctx.enter_context(tc.tile_pool(name="singles", bufs=1))

        id_bf = singles.tile([128, 128], BF)
        make_identity(nc, id_bf)
        id_f32 = singles.tile([128, 128], F32)
        make_identity(nc, id_f32)

        # Resident exp(mask^T): ET[p, tb, q] = exp(mask[q, tb*128 + p])
        ET = singles.tile([128, TB, SQ], BF)

        # PSUM budget (8 banks): sc 2x3 + pv 1 + small 1
        psc = ctx.enter_context(tc.tile_pool(name="psc", bufs=2, space="PSUM"))
        ppv = ctx.enter_context(tc.tile_pool(name="ppv", bufs=1, space="PSUM"))
        psm = ctx.enter_context(tc.tile_pool(name="psm", bufs=1, space="PSUM"))

        # ---------------- Phase A: build E^T ----------------
        # (mask pools scoped so their SBUF is released before phase B)
        r4 = min(4, TB)
        with tc.tile_pool(name="mpool", bufs=2) as mpool:
            for qb in range(QB):
                mfp = mpool.tile([128, SK], F32, tag="mfp")
                nc.sync.dma_start(out=mfp, in_=m_d[qb * 128:(qb + 1) * 128, :])
                mbf = mpool.tile([128, SK], BF, tag="mbf")
                nc.gpsimd.tensor_copy(out=mbf, in_=mfp)
                for half in range(max(1, TB // r4)):
                    mt_ps = psm.tile([128, r4, 128], BF, tag="small")
                    for c in range(r4):
                        tbn = half * r4 + c
                        nc.tensor.transpose(
                            mt_ps[:, c, :], mbf[:, tbn * 128:(tbn + 1) * 128], id_bf
                        )
                    nc.scalar.activation(
                        out=ET[:, half * r4:(half + 1) * r4, qb * 128:(qb + 1) * 128],
                        in_=mt_ps,
                        func=AF.Exp,
                    )

        kpool = ctx.enter_context(tc.tile_pool(name="kpool", bufs=2))
        ktpool = ctx.enter_context(tc.tile_pool(name="ktpool", bufs=2))
        qpool = ctx.enter_context(tc.tile_pool(name="qpool", bufs=2))
        vpool = ctx.enter_context(tc.tile_pool(name="vpool", bufs=2))
        attnp = ctx.enter_context(tc.tile_pool(name="attnp", bufs=3))
        outp = ctx.enter_context(tc.tile_pool(name="outp", bufs=2))

        # ---------------- Phase B: per-head flash ----------------
        for h in range(H):
            # K^T: load K fp32 (HWDGE), cast on gpsimd, PE-transpose -> kt
            kfp = kpool.tile([128, TB, D], F32, tag="kfp")
            nc.sync.dma_start(
                out=kfp, in_=k_d[h].rearrange("(b p) d -> p b d", p=128)
            )
            kbf = kpool.tile([128, TB, D], BF, tag="kbf")
            nc.gpsimd.tensor_copy(out=kbf, in_=kfp)
            kt = ktpool.tile([64, TB, 128], BF, tag="kt")
            for r in range(max(1, TB // r4)):
                ktps = psm.tile([64, r4, 128], BF, tag="small")
                for c in range(r4):
                    nc.tensor.transpose(ktps[:, c, :], kbf[:, r * r4 + c, :], id_bf)
                nc.vector.tensor_copy(out=kt[:, r * r4:(r + 1) * r4, :], in_=ktps)

            # Q^T: same
            qfp = qpool.tile([128, QB, D], F32, tag="qfp")
            nc.sync.dma_start(
                out=qfp, in_=q_d[h].rearrange("(b p) d -> p b d", p=128)
            )
            qbf = qpool.tile([128, QB, D], BF, tag="qbf")
            nc.gpsimd.tensor_copy(out=qbf, in_=qfp)
            qt = qpool.tile([64, QB, 128], BF, tag="qt")
            for r in range(max(1, QB // r4)):
                qtps = psm.tile([64, r4, 128], BF, tag="small")
                for c in range(r4):
                    nc.tensor.transpose(qtps[:, c, :], qbf[:, r * r4 + c, :], id_bf)
                nc.vector.tensor_copy(out=qt[:, r * r4:(r + 1) * r4, :], in_=qtps)

            # V with ones column appended (denominator rides along in PV)
            vfp = vpool.tile([128, TB, D], F32, tag="vfp")
            nc.sync.dma_start(
                out=vfp, in_=v_d[h].rearrange("(b p) d -> p b d", p=128)
            )
            v1 = vpool.tile([128, TB, D + 1], BF, tag="v1")
            nc.gpsimd.tensor_copy(out=v1[:, :, 0:D], in_=vfp)
            nc.gpsimd.memset(v1[:, :, D:D + 1], 1.0)

            for qp in range(QP):
                pv = ppv.tile([D + 1, QW], F32, tag="pv")
                nbatch = (TB + EB - 1) // EB
                for ib in range(nbatch):
                    tbs = list(range(ib * EB, min((ib + 1) * EB, TB)))
                    nb = len(tbs)
                    sc = psc.tile([128, EB, QW], F32, tag="sc")
                    for j, tb in enumerate(tbs):
                        nc.tensor.matmul(
                            sc[:, j, :],
                            kt[:, tb, :],
                            qt[:, qp * QC:(qp + 1) * QC, :],
                        )
                    attn = attnp.tile([128, EB, QW], BF, tag="attn")
                    nc.scalar.activation(
                        out=attn[:, :nb, :],
                        in_=sc[:, :nb, :],
                        func=AF.Exp,
                        scale=SCALE,
                    )
                    attnm = attnp.tile([128, EB, QW], BF, tag="attnm")
                    nc.vector.tensor_mul(
                        attnm[:, :nb, :],
                        attn[:, :nb, :],
                        ET[:, tbs[0]:tbs[0] + nb, qp * QW:(qp + 1) * QW],
                    )
                    for j, tb in enumerate(tbs):
                        nc.tensor.matmul(
                            pv,
                            v1[:, tb, :],
                            attnm[:, j, :],
                            start=(tb == 0),
                            stop=(tb == TB - 1),
                            skip_group_check=True,
                        )

                # Normalize + store: pv is out^T [65, 512] (row 64 = denom)
                pvs = outp.tile([D + 1, QW], F32, tag="pvs")
                nc.vector.tensor_copy(out=pvs, in_=pv)
                ot_ps = psm.tile([128, QC, D + 1], F32, tag="small")
                for cq in range(QC):
                    nc.tensor.transpose(
                        ot_ps[:, cq, :],
                        pvs[:, cq * 128:(cq + 1) * 128],
                        id_f32[0:D + 1, 0:D + 1],
                    )
                rden = outp.tile([128, QC], F32, tag="rden")
                nc.vector.reciprocal(out=rden, in_=ot_ps[:, :, D])
                osb = outp.tile([128, QC, D], F32, tag="osb")
                for cq in range(QC):
                    nc.vector.tensor_scalar_mul(
                        osb[:, cq, :], ot_ps[:, cq, 0:D], rden[:, cq:cq + 1]
                    )
                nc.scalar.dma_start(
                    out=o_d[h][qp * QW:(qp + 1) * QW, :].rearrange(
                        "(c p) d -> p c d", p=128
                    ),
                    in_=osb,
                )
    nc.compile()
    return nc


_NC_CACHE = None


def _get_nc():
    global _NC_CACHE
    if _NC_CACHE is None:
        _NC_CACHE = build_bass()
    return _NC_CACHE


def kernel(queries, keys, values, mask):
    queries = np.ascontiguousarray(np.asarray(queries), dtype=np.float32)
    keys = np.ascontiguousarray(np.asarray(keys), dtype=np.float32)
    values = np.ascontiguousarray(np.asarray(values), dtype=np.float32)
    mask = np.ascontiguousarray(np.asarray(mask), dtype=np.float32)

    nc = _get_nc()
    in_maps = []
    for c in range(N_CORES):
        hg, qg = divmod(c, 4)
        in_maps.append(
            {
                "queries": np.ascontiguousarray(
                    queries[hg * H:(hg + 1) * H, qg * SQ:(qg + 1) * SQ, :]
                ),
                "keys": np.ascontiguousarray(keys[hg * H:(hg + 1) * H]),
                "values": np.ascontiguousarray(values[hg * H:(hg + 1) * H]),
                "mask": np.ascontiguousarray(mask[qg * SQ:(qg + 1) * SQ, :]),
            }
        )
    results = bass2jax.run_bass_via_pjrt(nc, in_maps, n_cores=N_CORES)
    out = np.zeros((2 * H, 4 * SQ, D), np.float32)
    for c in range(N_CORES):
        hg, qg = divmod(c, 4)
        out[hg * H:(hg + 1) * H, qg * SQ:(qg + 1) * SQ, :] = results[c]["out"]
    return out
